# revision 1
# baseline (speedup 1.0000x reference)
"""GCNNet forward on 8 Trainium2 NeuronCores (Bass/Tile).

Sharding: nodes in 8 contiguous blocks (SHARD rows each, tail zero-padded);
edges assigned to the core owning their *destination*. Per conv layer:

  hw_pre = dinv * (BN(h) @ W)   -- BN folded into the weights (W' = diag(s)W,
                                   c = t@W); per-feature stats via PE
                                   ones-matmul partition reduction + a tiny
                                   AllReduce of [feat, 2] sums
  AllGather hw_pre -> hw_full   -- gather source, tile-layout rows
  per-edge messages come in via dma_gather (SWDGE custom ucode, int16
  indices, sources bucketed into 32768-row chunks)
  segment-sum on the PE: per 128-edge tile a one-hot matrix (DVE
  iota-compare against the dst slot) is matmul'd into a PSUM accumulator
  per (chunk, dst-block) group, then spilled into an SBUF accumulator that
  was seeded with the self-loop term (hw_pre itself)
  h_next = relu(dinv * agg + b) * valid

The GCN normalization is separable (norm_e = dinv[src] * dinv[dst] for kept
edges), so no per-edge float arithmetic runs on any compute engine — only
the gather DMA and the PE segment-sum. Pooling reuses the one-hot matmul
over batch ids + an AllReduce; the tiny 2-layer head runs redundantly on
every core.

All plain DMAs use nc.gpsimd (SWDGE): HWDGE (nc.sync) DMAs alongside the
custom SWDGE gather ucode crash the device (empirically bisected).
"""
import sys

sys.path.insert(0, "/opt/trn_rl_repo")

import numpy as np

import concourse.bacc as bacc
import concourse.mybir as mybir
import concourse.tile as tile

F32 = mybir.dt.float32
I16 = mybir.dt.int16

NCORES = 8
CHUNK = 32768          # gather-index range per int16 chunk
CALL_TILES = 64        # max 128-edge tiles per dma_gather call
OH_BATCH = 8           # tiles per DVE one-hot generation op
PAD_SLOT = 200         # one-hot slot for padding edges (matches nothing)
EPS = 1e-5
SCRATCH = 32768        # SWDGE descriptor carveout bytes/partition
REPEAT_MP = 1          # timing: repeat the message-passing phase per layer
ABLATE = ""            # timing: "gather_only" | "no_gather" | ""


def _wrap_idx(a):
    """int16 indices -> SWDGE layout [128, n/16] (16-wrapped, 8x replicated)."""
    assert a.size % 16 == 0
    w = a.reshape(-1, 16).T.copy()
    return np.ascontiguousarray(np.tile(w, (8, 1)))


def _tab128(a, nt):
    """[nt*128] -> [128, nt] tile-column table (node l -> [l%128, l//128])."""
    return np.ascontiguousarray(a.reshape(nt, 128).T)


def preprocess(inputs):
    x = np.asarray(inputs["x"], np.float32)
    ei = np.asarray(inputs["edge_index"], np.int64)
    batch = np.asarray(inputs["batch"], np.int64)
    N, F = x.shape
    W_conv = np.asarray(inputs["W_conv"], np.float32)
    H = W_conv.shape[-1]
    W_cls = np.asarray(inputs["W_cls"], np.float32)
    C = W_cls.shape[-1]
    G = int(np.asarray(inputs["num_graphs"]))
    assert G <= 128 and F <= 128 and H <= 128

    SHARD = -(-N // (NCORES * 128)) * 128
    NT = SHARD // 128
    NPAD = NCORES * SHARD
    NCHUNK = -(-NPAD // CHUNK)

    row, col = ei[0], ei[1]
    keep = row != col
    row = row[keep]
    col = col[keep]

    deg = (np.bincount(row, minlength=N) + 1).astype(np.float32)
    dinv = (np.float32(1.0) / np.sqrt(deg)).astype(np.float32)
    dinv_pad = np.zeros(NPAD, np.float32)
    valid_pad = np.zeros(NPAD, np.float32)
    batch_pad = np.full(NPAD, PAD_SLOT, np.int16)
    dinv_pad[:N] = dinv
    valid_pad[:N] = 1.0
    batch_pad[:N] = batch.astype(np.int16)

    # hw_full rows use tile-layout: node l = t*128 + p on core k sits at
    # global row k*SHARD + p*NT + t.
    nglob = np.arange(NPAD, dtype=np.int64)
    n_local = nglob % SHARD
    tl_row = (nglob // SHARD) * SHARD + (n_local % 128) * NT + n_local // 128

    src_row = tl_row[row]
    dst_core = col // SHARD
    dst_local = col % SHARD

    NBLK = NT
    per_core = []
    cnts = np.zeros((NCORES, NCHUNK * NBLK), np.int64)
    for k in range(NCORES):
        m = dst_core == k
        r = src_row[m]
        c = dst_local[m]
        key = (r // CHUNK) * NBLK + (c >> 7)
        order = np.argsort(key, kind="stable")
        per_core.append((r[order], c[order], key[order]))
        cnts[k] = np.bincount(key, minlength=NCHUNK * NBLK)

    # equalized tile counts per (chunk, block) across cores (shared schedule)
    gtiles = -(-cnts.max(axis=0) // 128)

    tile_block = []
    tile_first = []
    tile_last = []
    calls = []           # (chunk, tile_off, n_tiles)
    n_tiles = 0
    for ch in range(NCHUNK):
        ch_start = n_tiles
        for blk in range(NBLK):
            t = int(gtiles[ch * NBLK + blk])
            for j in range(t):
                tile_block.append(blk)
                tile_first.append(j == 0)
                tile_last.append(j == t - 1)
            n_tiles += t
        nt_ch = n_tiles - ch_start
        off = 0
        while off < nt_ch:
            n = min(CALL_TILES, nt_ch - off)
            calls.append((ch, ch_start + off, n))
            off += n
    TOT = n_tiles * 128

    goff = np.zeros(NCHUNK * NBLK + 1, np.int64)
    np.cumsum(gtiles * 128, out=goff[1:])
    src_tab = []
    slot_tab = []
    for k in range(NCORES):
        r, c, key = per_core[k]
        src_s = np.zeros(TOT, np.int64)          # pads gather row 0 of chunk
        slot_s = np.full(TOT, PAD_SLOT, np.int16)
        kcnt = cnts[k]
        start_of_group = np.concatenate([[0], np.cumsum(kcnt)[:-1]])
        within = np.arange(r.size, dtype=np.int64) - np.repeat(start_of_group, kcnt)
        pos = goff[key] + within
        src_s[pos] = r % CHUNK
        slot_s[pos] = (c & 127).astype(np.int16)
        src_tab.append(_wrap_idx(src_s.astype(np.int16)))
        slot_tab.append(np.ascontiguousarray(slot_s.reshape(n_tiles, 128).T))

    meta = dict(
        N=N, F=F, H=H, C=C, G=G, SHARD=SHARD, NT=NT, NPAD=NPAD,
        NCHUNK=NCHUNK, n_tiles=n_tiles,
        tile_block=tile_block, tile_first=tile_first, tile_last=tile_last,
        calls=calls,
    )

    params = dict(
        W_feat=np.ascontiguousarray(np.asarray(inputs["W_feat"], np.float32)),
        W_conv_cat=np.ascontiguousarray(
            W_conv.transpose(1, 0, 2).reshape(H, 3 * H)),
        W_fc=np.ascontiguousarray(np.asarray(inputs["W_fc"], np.float32)),
        W_cls=np.ascontiguousarray(W_cls),
        b_conv_rep=np.ascontiguousarray(np.broadcast_to(
            np.asarray(inputs["b_conv"], np.float32)[None, :, :], (128, 3, H))),
        g_conv=np.ascontiguousarray(np.asarray(inputs["bn_conv_g"], np.float32).T),
        b2_conv=np.ascontiguousarray(np.asarray(inputs["bn_conv_b"], np.float32).T),
        g_feat=np.asarray(inputs["bn_feat_g"], np.float32).reshape(F, 1).copy(),
        b2_feat=np.asarray(inputs["bn_feat_b"], np.float32).reshape(F, 1).copy(),
        g_fc=np.asarray(inputs["bn_fc_g"], np.float32).reshape(H, 1).copy(),
        b2_fc=np.asarray(inputs["bn_fc_b"], np.float32).reshape(H, 1).copy(),
        g_hid=np.asarray(inputs["bn_hidden_g"], np.float32).reshape(H, 1).copy(),
        b2_hid=np.asarray(inputs["bn_hidden_b"], np.float32).reshape(H, 1).copy(),
        b_fc=np.asarray(inputs["b_fc"], np.float32).reshape(1, H).copy(),
        b_cls=np.asarray(inputs["b_cls"], np.float32).reshape(1, C).copy(),
        identity=np.eye(128, dtype=np.float32),
        iota=np.ascontiguousarray(
            np.broadcast_to(np.arange(128, dtype=np.int16)[None, :], (128, 128))),
        ones_col=np.ones((128, 1), np.float32),
        ones_row=np.ones((1, 128), np.float32),
        gvalid=(np.arange(128) < G).astype(np.float32).reshape(128, 1),
        eps_col=np.full((128, 1), EPS, np.float32),
    )

    x_pad = np.zeros((NPAD, F), np.float32)
    x_pad[:N] = x
    in_maps = []
    for k in range(NCORES):
        sl = slice(k * SHARD, (k + 1) * SHARD)
        m = dict(params)
        m["x"] = np.ascontiguousarray(x_pad[sl])
        m["dinv_tab"] = _tab128(dinv_pad[sl], NT)
        m["valid_tab"] = _tab128(valid_pad[sl], NT)
        m["batch_tab"] = np.ascontiguousarray(batch_pad[sl].reshape(NT, 128).T)
        m["src_idx"] = src_tab[k]
        m["slot_tab"] = slot_tab[k]
        in_maps.append(m)

    return meta, in_maps


def build_program(meta):
    N, F, H, C, G = meta["N"], meta["F"], meta["H"], meta["C"], meta["G"]
    SHARD, NT, NPAD = meta["SHARD"], meta["NT"], meta["NPAD"]
    n_tiles = meta["n_tiles"]
    tile_block = meta["tile_block"]
    tile_first = meta["tile_first"]
    tile_last = meta["tile_last"]
    calls = meta["calls"]
    TOT = n_tiles * 128

    nc = bacc.Bacc("TRN2", target_bir_lowering=False, debug=False,
                   num_devices=NCORES, dynamic_dma_scratch_size=SCRATCH)

    x_d = nc.dram_tensor("x", [SHARD, F], F32, kind="ExternalInput")
    src_idx_d = nc.dram_tensor("src_idx", [128, TOT // 16], I16, kind="ExternalInput")
    slot_tab_d = nc.dram_tensor("slot_tab", [128, n_tiles], I16, kind="ExternalInput")
    batch_tab_d = nc.dram_tensor("batch_tab", [128, NT], I16, kind="ExternalInput")
    dinv_tab_d = nc.dram_tensor("dinv_tab", [128, NT], F32, kind="ExternalInput")
    valid_tab_d = nc.dram_tensor("valid_tab", [128, NT], F32, kind="ExternalInput")
    Wf_d = nc.dram_tensor("W_feat", [F, H], F32, kind="ExternalInput")
    Wc_d = nc.dram_tensor("W_conv_cat", [H, 3 * H], F32, kind="ExternalInput")
    Wfc_d = nc.dram_tensor("W_fc", [H, H], F32, kind="ExternalInput")
    Wcls_d = nc.dram_tensor("W_cls", [H, C], F32, kind="ExternalInput")
    bconv_d = nc.dram_tensor("b_conv_rep", [128, 3, H], F32, kind="ExternalInput")
    gconv_d = nc.dram_tensor("g_conv", [H, 3], F32, kind="ExternalInput")
    b2conv_d = nc.dram_tensor("b2_conv", [H, 3], F32, kind="ExternalInput")
    gfeat_d = nc.dram_tensor("g_feat", [F, 1], F32, kind="ExternalInput")
    b2feat_d = nc.dram_tensor("b2_feat", [F, 1], F32, kind="ExternalInput")
    gfc_d = nc.dram_tensor("g_fc", [H, 1], F32, kind="ExternalInput")
    b2fc_d = nc.dram_tensor("b2_fc", [H, 1], F32, kind="ExternalInput")
    ghid_d = nc.dram_tensor("g_hid", [H, 1], F32, kind="ExternalInput")
    b2hid_d = nc.dram_tensor("b2_hid", [H, 1], F32, kind="ExternalInput")
    bfc_d = nc.dram_tensor("b_fc", [1, H], F32, kind="ExternalInput")
    bcls_d = nc.dram_tensor("b_cls", [1, C], F32, kind="ExternalInput")
    ident_d = nc.dram_tensor("identity", [128, 128], F32, kind="ExternalInput")
    iota_d = nc.dram_tensor("iota", [128, 128], I16, kind="ExternalInput")
    onesc_d = nc.dram_tensor("ones_col", [128, 1], F32, kind="ExternalInput")
    onesr_d = nc.dram_tensor("ones_row", [1, 128], F32, kind="ExternalInput")
    gvalid_d = nc.dram_tensor("gvalid", [128, 1], F32, kind="ExternalInput")
    eps_d = nc.dram_tensor("eps_col", [128, 1], F32, kind="ExternalInput")
    out_d = nc.dram_tensor("out", [128, C], F32, kind="ExternalOutput")

    hwpre_d = nc.dram_tensor("hwpre_dram", [SHARD, H], F32, kind="Internal")
    hw_full = nc.dram_tensor("hw_full", [NPAD, H], F32, kind="Internal",
                             addr_space="Shared")
    statF_l = nc.dram_tensor("statF_l", [F, 2], F32, kind="Internal")
    statF_s = nc.dram_tensor("statF_s", [F, 2], F32, kind="Internal",
                             addr_space="Shared")
    statH_l = [nc.dram_tensor(f"statH_l{i}", [H, 2], F32, kind="Internal")
               for i in range(3)]
    statH_s = [nc.dram_tensor(f"statH_s{i}", [H, 2], F32, kind="Internal",
                              addr_space="Shared") for i in range(3)]
    hgp_d = nc.dram_tensor("hgp_dram", [128, H], F32, kind="Internal")
    hg_sh = nc.dram_tensor("hg_sh", [128, H], F32, kind="Internal",
                           addr_space="Shared")

    RG = [list(range(NCORES))]
    AF = mybir.ActivationFunctionType
    ALU = mybir.AluOpType
    inv_n = 1.0 / float(N)
    inv_g = 1.0 / float(G)

    with tile.TileContext(nc) as tc:
        with tc.tile_pool(name="per", bufs=1) as per, \
             tc.tile_pool(name="st", bufs=2) as st:
            slot_sb = per.tile([128, n_tiles], I16, tag="slots")
            dinv_sb = per.tile([128, NT], F32, tag="dinv")
            valid_sb = per.tile([128, NT], F32, tag="validt")
            batch_sb = per.tile([128, NT], I16, tag="batcht")
            iota_sb = per.tile([128, 128], I16, tag="iota")
            ident_sb = per.tile([128, 128], F32, tag="ident")
            onesc_sb = per.tile([128, 1], F32, tag="onesc")
            onesr_sb = per.tile([1, 128], F32, tag="onesr")
            gvalid_sb = per.tile([128, 1], F32, tag="gvalid")
            eps_sb = per.tile([128, 1], F32, tag="epsc")
            Wf_sb = per.tile([F, H], F32, tag="wf")
            Wc_sb = per.tile([H, 3 * H], F32, tag="wc")
            Wfc_sb = per.tile([H, H], F32, tag="wfc")
            Wcls_sb = per.tile([H, C], F32, tag="wcls")
            bconv_sb = per.tile([128, 3, H], F32, tag="bconv")
            gconv_sb = per.tile([H, 3], F32, tag="gconv")
            b2conv_sb = per.tile([H, 3], F32, tag="b2conv")
            gfeat_sb = per.tile([F, 1], F32, tag="gfeat")
            b2feat_sb = per.tile([F, 1], F32, tag="b2feat")
            gfc_sb = per.tile([H, 1], F32, tag="gfc")
            b2fc_sb = per.tile([H, 1], F32, tag="b2fc")
            ghid_sb = per.tile([H, 1], F32, tag="ghid")
            b2hid_sb = per.tile([H, 1], F32, tag="b2hid")
            bfc_sb = per.tile([1, H], F32, tag="bfc")
            bcls_sb = per.tile([1, C], F32, tag="bcls")

            for sb, d in [(slot_sb, slot_tab_d), (dinv_sb, dinv_tab_d),
                          (valid_sb, valid_tab_d), (batch_sb, batch_tab_d),
                          (iota_sb, iota_d), (ident_sb, ident_d),
                          (onesc_sb, onesc_d), (onesr_sb, onesr_d),
                          (gvalid_sb, gvalid_d), (eps_sb, eps_d),
                          (Wf_sb, Wf_d), (Wc_sb, Wc_d), (Wfc_sb, Wfc_d),
                          (Wcls_sb, Wcls_d), (bconv_sb, bconv_d),
                          (gconv_sb, gconv_d), (b2conv_sb, b2conv_d),
                          (gfeat_sb, gfeat_d), (b2feat_sb, b2feat_d),
                          (gfc_sb, gfc_d), (b2fc_sb, b2fc_d),
                          (ghid_sb, ghid_d), (b2hid_sb, b2hid_d),
                          (bfc_sb, bfc_d), (bcls_sb, bcls_d)]:
                nc.gpsimd.dma_start(sb[:], d[:])

            # ------------- helpers -------------
            def stats_reduce(get_tile, nt_count, Win, dst_res):
                """Per-feature [Win,2] sum/sumsq over node tiles via PE."""
                with tc.tile_pool(name="stp", bufs=1, space="PSUM") as stp:
                    ps_sum = stp.tile([Win, 1], F32, tag="pssum")
                    ps_sq = stp.tile([Win, 1], F32, tag="pssq")
                    for t in range(nt_count):
                        src = get_tile(t)
                        sq = st.tile([128, Win], F32, tag="sqb")
                        nc.scalar.activation(sq[:], src, AF.Square)
                        nc.tensor.matmul(ps_sum[:], src, onesc_sb[:],
                                         start=(t == 0), stop=(t == nt_count - 1))
                        nc.tensor.matmul(ps_sq[:], sq[:], onesc_sb[:],
                                         start=(t == 0), stop=(t == nt_count - 1))
                    nc.vector.tensor_copy(dst_res[:, 0:1], ps_sum[:])
                    nc.vector.tensor_copy(dst_res[:, 1:2], ps_sq[:])

            def stats_allreduce(get_tile, nt_count, Win, out_l, out_s):
                res = st.tile([128, 2], F32, tag="statres", name="statres")[:Win, :]
                stats_reduce(get_tile, nt_count, Win, res)
                nc.gpsimd.dma_start(out_l[:], res)
                nc.gpsimd.collective_compute(
                    "AllReduce", ALU.add, replica_groups=RG,
                    ins=[out_l[:]], outs=[out_s[:]])
                gst = st.tile([128, 2], F32, tag="statg", name="statg")[:Win, :]
                nc.gpsimd.dma_start(gst, out_s[:])
                return gst

            def bn_fold(stats_sb, g_sb, b_sb, inv_count, W_sb, Win, Wout,
                        extra_bias=None):
                """stats [Win,2] -> W' = diag(s)@W and c = t@W (+extra)."""
                mean = st.tile([128, 1], F32, tag="bnm", name="bnm")[:Win, :]
                msq = st.tile([128, 1], F32, tag="bnq", name="bnq")[:Win, :]
                var = st.tile([128, 1], F32, tag="bnv", name="bnv")[:Win, :]
                sd = st.tile([128, 1], F32, tag="bnsd", name="bnsd")[:Win, :]
                s = st.tile([128, 1], F32, tag="bns", name="bns")[:Win, :]
                t = st.tile([128, 1], F32, tag="bnt", name="bnt")[:Win, :]
                Wp = st.tile([128, Wout], F32, tag="bnw", name="bnw")[:Win, :]
                c_sb = st.tile([1, Wout], F32, tag="bnc")
                nc.scalar.activation(mean, stats_sb[:, 0:1], AF.Copy,
                                     scale=float(inv_count))
                nc.scalar.activation(msq, stats_sb[:, 1:2], AF.Copy,
                                     scale=float(inv_count))
                nc.vector.tensor_mul(var, mean, mean)
                nc.vector.tensor_sub(var, msq, var)
                nc.scalar.activation(sd, var, AF.Sqrt, bias=eps_sb[:Win, :])
                nc.vector.reciprocal(s, sd)
                nc.vector.tensor_mul(s, s, g_sb)
                nc.vector.tensor_mul(t, mean, s)
                nc.vector.tensor_sub(t, b_sb, t)
                nc.vector.tensor_scalar_mul(Wp, W_sb, s)
                with tc.tile_pool(name="bnp", bufs=1, space="PSUM") as bnp:
                    c_ps = bnp.tile([1, Wout], F32, tag="bncp")
                    nc.tensor.matmul(c_ps[:], t, W_sb, start=True, stop=True)
                    if extra_bias is not None:
                        nc.vector.tensor_add(c_sb[:], c_ps[:], extra_bias)
                    else:
                        nc.vector.tensor_copy(c_sb[:], c_ps[:])
                return Wp, c_sb

            # ------------- layer 0: h0 = relu(BN(x) @ W_feat) -------------
            def x_tile(t):
                xt = st.tile([128, F], F32, tag="xt")
                nc.gpsimd.dma_start(xt[:], x_d[t * 128:(t + 1) * 128, :])
                return xt[:]

            gstF = stats_allreduce(x_tile, NT, F, statF_l, statF_s)
            WpF, cF = bn_fold(gstF, gfeat_sb, b2feat_sb, inv_n, Wf_sb, F, H)
            h_sb = per.tile([128, NT, H], F32, tag="h")
            with tc.tile_pool(name="l0p", bufs=2, space="PSUM") as l0p, \
                 tc.tile_pool(name="l0s", bufs=2) as l0s:
                for t in range(NT):
                    xt = x_tile(t)
                    tp = l0p.tile([F, 128], F32, tag="l0T")
                    nc.tensor.transpose(tp[:], xt, ident_sb[:])
                    xT = l0s.tile([F, 128], F32, tag="l0hT")
                    nc.scalar.copy(xT[:], tp[:])
                    ps = l0p.tile([128, H], F32, tag="l0mm")
                    nc.tensor.matmul(ps[:], xT[:], WpF, start=True, stop=False)
                    nc.tensor.matmul(ps[:], onesr_sb[:], cF[:],
                                     start=False, stop=True)
                    nc.scalar.activation(h_sb[:, t, :], ps[:], AF.Relu)
            nc.vector.tensor_tensor(
                h_sb[:], h_sb[:],
                valid_sb[:].unsqueeze(2).broadcast_to([128, NT, H]), ALU.mult)

            # ------------- conv layers -------------
            hwpre_sb = per.tile([128, NT, H], F32, tag="hwpre")
            agg_sb = per.tile([128, NT, H], F32, tag="agg")
            for li in range(3):
                gstH = stats_allreduce(lambda t: h_sb[:, t, :], NT, H,
                                       statH_l[li], statH_s[li])
                WpH, cH = bn_fold(gstH, gconv_sb[:, li:li + 1],
                                  b2conv_sb[:, li:li + 1], inv_n,
                                  Wc_sb[:, li * H:(li + 1) * H], H, H)
                with tc.tile_pool(name="tfp", bufs=2, space="PSUM") as tfp, \
                     tc.tile_pool(name="tfs", bufs=2) as tfs:
                    for t in range(NT):
                        tp = tfp.tile([H, 128], F32, tag="tpT")
                        nc.tensor.transpose(tp[:], h_sb[:, t, :], ident_sb[:])
                        hT = tfs.tile([H, 128], F32, tag="hT")
                        nc.scalar.copy(hT[:], tp[:])
                        ps = tfp.tile([128, H], F32, tag="tpmm")
                        nc.tensor.matmul(ps[:], hT[:], WpH, start=True, stop=False)
                        nc.tensor.matmul(ps[:], onesr_sb[:], cH[:],
                                         start=False, stop=True)
                        nc.vector.tensor_scalar_mul(
                            hwpre_sb[:, t, :], ps[:], dinv_sb[:, t:t + 1])
                nc.gpsimd.dma_start(
                    hwpre_d[:].rearrange("(p t) e -> p t e", t=NT), hwpre_sb[:])
                nc.gpsimd.collective_compute(
                    "AllGather", ALU.bypass, replica_groups=RG,
                    ins=[hwpre_d[:]], outs=[hw_full[:]])
                nc.vector.tensor_copy(agg_sb[:], hwpre_sb[:])

                with tc.tile_pool(name="mp", bufs=2) as mp, \
                     tc.tile_pool(name="mpp", bufs=3, space="PSUM") as mpp:
                  ps_cur = None
                  for _rep in range(REPEAT_MP):
                    for (ch, tile_off, ntl) in calls:
                          nidx = ntl * 128
                          col0 = tile_off * 8
                          idx_sb = mp.tile([128, CALL_TILES * 8], I16, tag="idx")
                          nc.gpsimd.dma_start(
                              idx_sb[:, :ntl * 8],
                              src_idx_d[:, col0:col0 + ntl * 8])
                          msg = mp.tile([128, CALL_TILES, H], F32, tag="msg")
                          rows = min(CHUNK, NPAD - ch * CHUNK)
                          if ABLATE != "no_gather":
                              nc.gpsimd.dma_gather(
                                  out_ap=msg[:, :ntl, :],
                                  in_ap=hw_full[ch * CHUNK: ch * CHUNK + rows, :],
                                  idxs_ap=idx_sb[:, :ntl * 8],
                                  num_idxs=nidx, num_idxs_reg=nidx, elem_size=H,
                                  single_packet=False)
                          if ABLATE == "gather_only":
                              continue
                          for b0 in range(0, ntl, OH_BATCH):
                              nb = min(OH_BATCH, ntl - b0)
                              gt0 = tile_off + b0
                              S = mp.tile([128, OH_BATCH, 128], F32, tag="oneh")
                              nc.vector.tensor_tensor(
                                  S[:, :nb, :],
                                  slot_sb[:, gt0:gt0 + nb].unsqueeze(2)
                                  .broadcast_to([128, nb, 128]),
                                  iota_sb[:].unsqueeze(1)
                                  .broadcast_to([128, nb, 128]),
                                  ALU.is_equal)
                              for j in range(nb):
                                  gt = gt0 + j
                                  if tile_first[gt]:
                                      ps_cur = mpp.tile([128, H], F32, tag="aggps")
                                  nc.tensor.matmul(
                                      ps_cur[:], S[:, j, :], msg[:, b0 + j, :],
                                      start=tile_first[gt], stop=tile_last[gt])
                                  if tile_last[gt]:
                                      blk = tile_block[gt]
                                      nc.vector.tensor_add(
                                          agg_sb[:, blk, :], agg_sb[:, blk, :],
                                          ps_cur[:])

                nc.vector.tensor_tensor(
                    agg_sb[:], agg_sb[:],
                    dinv_sb[:].unsqueeze(2).broadcast_to([128, NT, H]),
                    ALU.mult)
                nc.vector.tensor_tensor(
                    agg_sb[:], agg_sb[:],
                    bconv_sb[:, li, :].unsqueeze(1).broadcast_to([128, NT, H]),
                    ALU.add)
                h_sb = per.tile([128, NT, H], F32, tag="h")
                nc.scalar.activation(h_sb[:], agg_sb[:], AF.Relu)
                nc.vector.tensor_tensor(
                    h_sb[:], h_sb[:],
                    valid_sb[:].unsqueeze(2).broadcast_to([128, NT, H]),
                    ALU.mult)

            # ------------- pooling -------------
            with tc.tile_pool(name="plp", bufs=1, space="PSUM") as plp, \
                 tc.tile_pool(name="pls", bufs=2) as pls:
                ps_hg = plp.tile([128, H], F32, tag="pshg")
                for t0 in range(0, NT, OH_BATCH):
                    nb = min(OH_BATCH, NT - t0)
                    S = pls.tile([128, OH_BATCH, 128], F32, tag="poneh")
                    nc.vector.tensor_tensor(
                        S[:, :nb, :],
                        batch_sb[:, t0:t0 + nb].unsqueeze(2)
                        .broadcast_to([128, nb, 128]),
                        iota_sb[:].unsqueeze(1).broadcast_to([128, nb, 128]),
                        ALU.is_equal)
                    for j in range(nb):
                        t = t0 + j
                        nc.tensor.matmul(ps_hg[:], S[:, j, :], h_sb[:, t, :],
                                         start=(t == 0), stop=(t == NT - 1))
                hgp_sb = pls.tile([128, H], F32, tag="hgp")
                nc.vector.tensor_copy(hgp_sb[:], ps_hg[:])
                nc.gpsimd.dma_start(hgp_d[:], hgp_sb[:])
            nc.gpsimd.collective_compute(
                "AllReduce", ALU.add, replica_groups=RG,
                ins=[hgp_d[:]], outs=[hg_sh[:]])

            # ------------- head (redundant on every core) -------------
            with tc.tile_pool(name="hd", bufs=1) as hd, \
                 tc.tile_pool(name="hdp", bufs=1, space="PSUM") as hdp:
                hg = hd.tile([128, H], F32, tag="hg")
                nc.gpsimd.dma_start(hg[:], hg_sh[:])
                stat2 = st.tile([128, 2], F32, tag="stat2", name="stat2")[:H, :]
                stats_reduce(lambda t: hg[:], 1, H, stat2)
                Wp2, c2 = bn_fold(stat2, gfc_sb, b2fc_sb, inv_g, Wfc_sb, H, H,
                                  extra_bias=bfc_sb[:])
                tp = hdp.tile([H, 128], F32, tag="hdT")
                nc.tensor.transpose(tp[:], hg[:, :H], ident_sb[:])
                hgT = hd.tile([H, 128], F32, tag="hgT")
                nc.scalar.copy(hgT[:], tp[:])
                ps2 = hdp.tile([128, H], F32, tag="hdmm")
                nc.tensor.matmul(ps2[:], hgT[:], Wp2, start=True, stop=False)
                nc.tensor.matmul(ps2[:], onesr_sb[:], c2[:], start=False, stop=True)
                hg2 = hd.tile([128, H], F32, tag="hg2")
                nc.scalar.activation(hg2[:], ps2[:], AF.Relu)
                nc.vector.tensor_scalar_mul(hg2[:], hg2[:], gvalid_sb[:])

                stat3 = st.tile([128, 2], F32, tag="stat3", name="stat3")[:H, :]
                stats_reduce(lambda t: hg2[:], 1, H, stat3)
                Wp3, c3 = bn_fold(stat3, ghid_sb, b2hid_sb, inv_g, Wcls_sb, H, C,
                                  extra_bias=bcls_sb[:])
                tp2 = hdp.tile([H, 128], F32, tag="hdT2")
                nc.tensor.transpose(tp2[:], hg2[:, :H], ident_sb[:])
                hg2T = hd.tile([H, 128], F32, tag="hg2T")
                nc.scalar.copy(hg2T[:], tp2[:])
                ps3 = hdp.tile([128, C], F32, tag="hdmm2")
                nc.tensor.matmul(ps3[:], hg2T[:], Wp3, start=True, stop=False)
                nc.tensor.matmul(ps3[:], onesr_sb[:], c3[:], start=False, stop=True)
                out_sb = hd.tile([128, C], F32, tag="outsb")
                nc.vector.tensor_copy(out_sb[:], ps3[:])
                nc.gpsimd.dma_start(out_d[:], out_sb[:])

    nc.compile()
    return nc


def build_all(inputs):
    meta, in_maps = preprocess(inputs)
    nc = build_program(meta)
    return nc, meta, in_maps


def kernel(**inputs):
    from concourse import bass_utils
    nc, meta, in_maps = build_all(inputs)
    res = bass_utils.run_bass_kernel_spmd(
        nc, in_maps, core_ids=list(range(NCORES)))
    out = np.asarray(res.results[0]["out"], np.float32)
    return np.ascontiguousarray(out[:meta["G"], :])

